# revision 1
# baseline (speedup 1.0000x reference)
"""Trainium2 Bass kernel for nn_Conv2DLayer_16011638080159.

Math: out = C * (x @ weight.sum(0))   with x [524288, 512], weight [9, 512].
Equivalent to a row-wise dot product of x with w_eff = C * weight.sum(0).

Strategy (pure data parallel, per sharding hint):
  - Shard x along the batch axis across 8 NeuronCores (65536 rows each).
  - Host-side prep: fold the tiny K=9 weight sum and the C scale into a
    single [C] vector, replicated to a [128, 8*C] SBUF-ready constant.
  - Per core: stream x in [128 partitions, 8 rows x 512] tiles from HBM
    with 6-deep buffering, alternating the two HWDGE rings. The kernel is
    HBM bound (~415 us/core pure-DMA floor measured at 8 cores), so the
    row-dot-products are split so each compute engine stays below that:
      * Vector engine: fp32 tensor_tensor multiply of the whole tile by
        the replicated weight (1x mode), plus a segmented tensor_reduce
        for 1 of the 8 rows  (~320 us/core busy).
      * Scalar engine: the other 7 rows via ACTIVATE(Copy, accum_out),
        which sums 512 elems/row at 1 elem/cycle (~355 us/core busy).
  - Row mapping: shard row (p*512 + t*R + r) sits at partition p, tile t,
    slot r, so the per-core result tile [128, 512] is exactly the row-major
    view of the per-core output [65536]; one contiguous DMA writes it out.
"""

import numpy as np

import concourse.bacc as bacc
import concourse.bass as bass
import concourse.tile as tile
from concourse import mybir
from concourse.bass_utils import run_bass_kernel_spmd

B = 524288        # total rows
C = 512           # row length
N_CORES = 8
BS = B // N_CORES  # 65536 rows per core
P = 128            # SBUF partitions
RPP = BS // P      # 512 rows per partition
R = 8              # rows per partition per tile
F = R * C          # 4096 free elems per tile
NT = RPP // R      # 64 tiles per core
K_DVE = 1          # rows per tile reduced on DVE via segmented tensor_reduce

_NC_CACHE = None
LAST_RESULT = None  # BassKernelResults of the most recent run (for profiling)


def _build() -> bass.Bass:
    # Bacc (not raw Bass): its compile() pass splits multi-sem waits into
    # EventSemaphore instructions -- the TRN2 ISA allows only 1 wait/inst.
    nc = bacc.Bacc(None, target_bir_lowering=False, debug=False)
    x = nc.dram_tensor("x", [BS, C], mybir.dt.float32, kind="ExternalInput")
    w = nc.dram_tensor("w", [P, F], mybir.dt.float32, kind="ExternalInput")
    out = nc.dram_tensor("out", [BS], mybir.dt.float32, kind="ExternalOutput")

    # shard row (p*RPP + t*R + r) -> partition p, tile t, free slot (r, c)
    xv = x.rearrange("(p t r) c -> t p (r c)", p=P, t=NT, r=R)
    ov = out.rearrange("(p f) -> p f", p=P)

    n_act = R - K_DVE  # rows per tile reduced on the Scalar engine

    with tile.TileContext(nc) as tc:
        with (
            tc.tile_pool(name="const", bufs=1) as cpool,
            tc.tile_pool(name="xs", bufs=6) as xs,
            tc.tile_pool(name="ys", bufs=4) as ys,
            tc.tile_pool(name="scr", bufs=2) as scr,
            tc.tile_pool(name="res", bufs=1) as res,
        ):
            w_t = cpool.tile([P, F], mybir.dt.float32)
            nc.sync.dma_start(out=w_t[:], in_=w[:, :])
            o_t = res.tile([P, RPP], mybir.dt.float32)
            for t in range(NT):
                # All x DMAs go on the SP HWDGE ring: SP has no compute, so
                # DMA issue is never queued behind engine work (issuing from
                # nc.scalar stalls the DMA behind pending ACTIVATEs).
                x_t = xs.tile([P, F], mybir.dt.float32)
                nc.sync.dma_start(out=x_t[:], in_=xv[t])

                # one fp32 TT multiply for the whole tile
                y_t = ys.tile([P, F], mybir.dt.float32)
                nc.vector.tensor_mul(y_t[:], x_t[:], w_t[:])

                # ACT accumulates rows K_DVE..R-1 (one 512-sum per row)
                for r in range(n_act):
                    s_t = scr.tile([P, C], mybir.dt.float32, tag="act_s")
                    col = t * R + K_DVE + r
                    nc.scalar.activation(
                        out=s_t[:],
                        in_=y_t[:, (K_DVE + r) * C:(K_DVE + r + 1) * C],
                        func=mybir.ActivationFunctionType.Copy,
                        accum_out=o_t[:, col: col + 1],
                    )

                # DVE reduces rows 0..K_DVE-1 in one segmented reduce
                nc.vector.tensor_reduce(
                    out=o_t[:, t * R: t * R + K_DVE],
                    in_=y_t[:, 0:K_DVE * C].rearrange("p (r c) -> p r c", c=C),
                    axis=mybir.AxisListType.X,
                    op=mybir.AluOpType.add,
                )
            nc.sync.dma_start(out=ov, in_=o_t[:])
    nc.finalize()
    return nc


def kernel(x: np.ndarray, weight: np.ndarray) -> np.ndarray:
    global _NC_CACHE, LAST_RESULT
    x = np.ascontiguousarray(np.asarray(x), dtype=np.float32)
    weight = np.asarray(weight, dtype=np.float32)

    w_eff = (C * weight.sum(axis=0)).astype(np.float32)   # [C]
    w_rep = np.ascontiguousarray(np.tile(w_eff, (P, R)))  # [P, F]

    if _NC_CACHE is None:
        _NC_CACHE = _build()

    in_maps = [
        {"x": x[i * BS:(i + 1) * BS], "w": w_rep} for i in range(N_CORES)
    ]
    LAST_RESULT = run_bass_kernel_spmd(
        _NC_CACHE, in_maps, core_ids=list(range(N_CORES))
    )
    return np.concatenate([r["out"] for r in LAST_RESULT.results])



# revision 5
# speedup vs baseline: 1.0135x; 1.0135x over previous
"""Trainium2 Bass kernel for nn_Conv2DLayer_16011638080159.

Math: out = C * (x @ weight.sum(0))   with x [524288, 512], weight [9, 512].
Equivalent to a row-wise dot product of x with w_eff = C * weight.sum(0).

Strategy (pure data parallel, per sharding hint):
  - Shard x along the batch axis across 8 NeuronCores (65536 rows each).
  - Host-side prep: fold the K=9 weight sum and the C scale into a single
    [C] vector, replicated to a [128, C] SBUF constant (256 KiB, loaded once
    on the Activation ring so the SP ring starts streaming x immediately).
  - Per core: stream x in [128, 8*C] tiles (16 KiB descriptors - measured
    fastest: 32 KiB descriptors run 2x slower per DMA engine). Even tiles
    ride the SP HWDGE ring, odd tiles the Activation HWDGE ring. Measured
    pure-DMA floor for this config: ~403 us/core (334 GB/s, the per-core
    DMA fabric cap; queue count does not change it).
  - Compute: DVE only, one fused affine_mul_reduce per row slot
    (out=scratch, accum_out=o column): accum = sum((x*1+0)*w) does the
    multiply AND the row-sum in one 512-elem pass, measured 613 ns/slot
    pipelined (the 82 ns accumulator read overlaps the next op) ->
    4.9 us/tile, under the ~5.2-6.3 us/tile DMA cadence.
    (tensor_tensor_reduce and scalar_tensor_tensor+accum both fail this
    compiler/runtime; affine_mul_reduce is the custom-DVE path that works.)
    The Activation engine does NO compute (its ~1 us/slot accumulate chain
    was the baseline's critical path at 453 us busy).
  - Row mapping: shard row (p*512 + t*R + r) sits at partition p, tile t,
    slot r, so the per-core result tile [128, 512] is exactly the row-major
    view of the per-core output [65536]; one contiguous DMA writes it out.
"""

import numpy as np

import concourse.bacc as bacc
import concourse.bass as bass
import concourse.tile as tile
from concourse import mybir
from concourse.bass_utils import run_bass_kernel_spmd

B = 524288        # total rows
C = 512           # row length
N_CORES = 8
BS = B // N_CORES  # 65536 rows per core
P = 128            # SBUF partitions
RPP = BS // P      # 512 rows per partition
R = 8              # rows per partition per tile -> 16 KiB descriptors
F = R * C          # 4096 free elems per tile
NT = RPP // R      # 64 tiles per core
XS_BUFS = 8        # x tile pool depth (8 * 2 MiB = 16 MiB SBUF)

_NC_CACHE = None
LAST_RESULT = None  # BassKernelResults of the most recent run (for profiling)


def _build() -> bass.Bass:
    # Bacc (not raw Bass): its compile() pass splits multi-sem waits into
    # EventSemaphore instructions -- the TRN2 ISA allows only 1 wait/inst.
    nc = bacc.Bacc(None, target_bir_lowering=False, debug=False)
    x = nc.dram_tensor("x", [BS, C], mybir.dt.float32, kind="ExternalInput")
    w = nc.dram_tensor("w", [P, C], mybir.dt.float32, kind="ExternalInput")
    out = nc.dram_tensor("out", [BS], mybir.dt.float32, kind="ExternalOutput")

    # shard row (p*RPP + t*R + r) -> partition p, tile t, free slot (r, c)
    xv = x.rearrange("(p t r) c -> t p (r c)", p=P, t=NT, r=R)
    ov = out.rearrange("(p f) -> p f", p=P)

    with tile.TileContext(nc) as tc:
        with (
            tc.tile_pool(name="const", bufs=1) as cpool,
            tc.tile_pool(name="xs", bufs=XS_BUFS) as xs,
            tc.tile_pool(name="dmv", bufs=2) as dmv,
            tc.tile_pool(name="res", bufs=1) as res,
        ):
            w_t = cpool.tile([P, C], mybir.dt.float32)
            # weight rides the Act ring, whose first x tile is tile 1, so
            # the SP ring starts streaming tile 0 with zero delay.
            nc.scalar.dma_start(out=w_t[:], in_=w[:, :])
            o_t = res.tile([P, RPP], mybir.dt.float32)
            for t in range(NT):
                x_t = xs.tile([P, F], mybir.dt.float32)
                if t % 2 == 0:
                    nc.sync.dma_start(out=x_t[:], in_=xv[t])
                else:
                    nc.scalar.dma_start(out=x_t[:], in_=xv[t])

                # DVE: fused (x*w) + row-sum, one slot per instruction
                for r in range(R):
                    col = t * R + r
                    dm = dmv.tile([P, C], mybir.dt.float32, tag="dve_scr")
                    nc.vector.affine_mul_reduce(
                        out=dm[:],
                        accum_out=o_t[:, col: col + 1],
                        in0=x_t[:, r * C:(r + 1) * C],
                        in1=w_t[:],
                        scale=1.0,
                        bias=0.0,
                    )
            nc.sync.dma_start(out=ov, in_=o_t[:])
    nc.finalize()
    return nc


def kernel(x: np.ndarray, weight: np.ndarray) -> np.ndarray:
    global _NC_CACHE, LAST_RESULT
    x = np.ascontiguousarray(np.asarray(x), dtype=np.float32)
    weight = np.asarray(weight, dtype=np.float32)

    w_eff = (C * weight.sum(axis=0)).astype(np.float32)   # [C]
    w_rep = np.ascontiguousarray(np.tile(w_eff, (P, 1)))  # [P, C]

    if _NC_CACHE is None:
        _NC_CACHE = _build()

    in_maps = [
        {"x": x[i * BS:(i + 1) * BS], "w": w_rep} for i in range(N_CORES)
    ]
    LAST_RESULT = run_bass_kernel_spmd(
        _NC_CACHE, in_maps, core_ids=list(range(N_CORES))
    )
    return np.concatenate([r["out"] for r in LAST_RESULT.results])


# revision 6
# speedup vs baseline: 1.0372x; 1.0233x over previous
"""Trainium2 Bass kernel for nn_Conv2DLayer_16011638080159.

Math: out = C * (x @ weight.sum(0))   with x [524288, 512], weight [9, 512].
Equivalent to a row-wise dot product of x with w_eff = C * weight.sum(0).

Strategy (pure data parallel, per sharding hint):
  - Shard x along the batch axis across 8 NeuronCores (65536 rows each).
  - Host-side prep: fold the K=9 weight sum and the C scale into a single
    [C] vector, replicated to a [128, C] SBUF constant (256 KiB, loaded once
    on the Activation ring so the SP ring starts streaming x immediately).
  - Per core: stream x in [128, 8*C] tiles (16 KiB descriptors - measured
    fastest: 32 KiB descriptors run 2x slower per DMA engine). Even tiles
    ride the SP HWDGE ring, odd tiles the Activation HWDGE ring. Measured
    pure-DMA floor for this config: ~403 us/core (334 GB/s, the per-core
    DMA fabric cap; queue count does not change it).
  - Compute: DVE only, one fused affine_mul_reduce per row slot
    (out=scratch, accum_out=o column): accum = sum((x*1+0)*w) does the
    multiply AND the row-sum in one 512-elem pass, measured 613 ns/slot
    pipelined (the 82 ns accumulator read overlaps the next op) ->
    4.9 us/tile, under the ~5.2-6.3 us/tile DMA cadence.
    (tensor_tensor_reduce and scalar_tensor_tensor+accum both fail this
    compiler/runtime; affine_mul_reduce is the custom-DVE path that works.)
    The Activation engine does NO compute (its ~1 us/slot accumulate chain
    was the baseline's critical path at 453 us busy).
  - Row mapping: shard row (p*512 + t*R + r) sits at partition p, tile t,
    slot r, so the per-core result tile [128, 512] is exactly the row-major
    view of the per-core output [65536]; one contiguous DMA writes it out.
"""

import numpy as np

import concourse.bacc as bacc
import concourse.bass as bass
import concourse.tile as tile
from concourse import mybir
from concourse.bass_utils import run_bass_kernel_spmd

B = 524288        # total rows
C = 512           # row length
N_CORES = 8
BS = B // N_CORES  # 65536 rows per core
P = 128            # SBUF partitions
RPP = BS // P      # 512 rows per partition
R = 8              # rows per partition per tile -> 16 KiB descriptors
F = R * C          # 4096 free elems per tile
NT = RPP // R      # 64 tiles per core
XS_BUFS = 8        # x tile pool depth (8 * 2 MiB = 16 MiB SBUF)

_NC_CACHE = None
LAST_RESULT = None  # BassKernelResults of the most recent run (for profiling)


def _build() -> bass.Bass:
    # Bacc (not raw Bass): its compile() pass splits multi-sem waits into
    # EventSemaphore instructions -- the TRN2 ISA allows only 1 wait/inst.
    nc = bacc.Bacc(None, target_bir_lowering=False, debug=False)
    x = nc.dram_tensor("x", [BS, C], mybir.dt.float32, kind="ExternalInput")
    w = nc.dram_tensor("w", [P, C], mybir.dt.float32, kind="ExternalInput")
    out = nc.dram_tensor("out", [BS], mybir.dt.float32, kind="ExternalOutput")

    # shard row (p*RPP + t*R + r) -> partition p, tile t, free slot (r, c)
    xv = x.rearrange("(p t r) c -> t p (r c)", p=P, t=NT, r=R)
    ov = out.rearrange("(p f) -> p f", p=P)

    with tile.TileContext(nc) as tc:
        with (
            tc.tile_pool(name="const", bufs=1) as cpool,
            tc.tile_pool(name="xs", bufs=XS_BUFS) as xs,
            tc.tile_pool(name="dmv", bufs=2) as dmv,
            tc.tile_pool(name="res", bufs=1) as res,
        ):
            w_t = cpool.tile([P, C], mybir.dt.float32)
            # weight rides the idle GpSimd SWDGE queue so BOTH HWDGE rings
            # start streaming x tiles immediately.
            nc.gpsimd.dma_start(out=w_t[:], in_=w[:, :])
            o_t = res.tile([P, RPP], mybir.dt.float32)
            # Output writeback is split: the bulk chunk (tiles 0..NT-9) goes
            # out on the idle GpSimd SWDGE as soon as those columns are done
            # (Pool just blocks on the sem - harmless), leaving only a tiny
            # 64-column chunk serialized after the last tile's compute.
            cut = (NT - 8) * R  # 448 columns
            for t in range(NT):
                x_t = xs.tile([P, F], mybir.dt.float32)
                if t % 2 == 0:
                    nc.sync.dma_start(out=x_t[:], in_=xv[t])
                else:
                    nc.scalar.dma_start(out=x_t[:], in_=xv[t])

                # DVE: fused (x*w) + row-sum, one slot per instruction
                for r in range(R):
                    col = t * R + r
                    dm = dmv.tile([P, C], mybir.dt.float32, tag="dve_scr")
                    nc.vector.affine_mul_reduce(
                        out=dm[:],
                        accum_out=o_t[:, col: col + 1],
                        in0=x_t[:, r * C:(r + 1) * C],
                        in1=w_t[:],
                        scale=1.0,
                        bias=0.0,
                    )
                if t == NT - 9:
                    nc.gpsimd.dma_start(out=ov[:, 0:cut], in_=o_t[:, 0:cut])
            nc.sync.dma_start(out=ov[:, cut:RPP], in_=o_t[:, cut:RPP])
    nc.finalize()
    return nc


def kernel(x: np.ndarray, weight: np.ndarray) -> np.ndarray:
    global _NC_CACHE, LAST_RESULT
    x = np.ascontiguousarray(np.asarray(x), dtype=np.float32)
    weight = np.asarray(weight, dtype=np.float32)

    w_eff = (C * weight.sum(axis=0)).astype(np.float32)   # [C]
    w_rep = np.ascontiguousarray(np.tile(w_eff, (P, 1)))  # [P, C]

    if _NC_CACHE is None:
        _NC_CACHE = _build()

    in_maps = [
        {"x": x[i * BS:(i + 1) * BS], "w": w_rep} for i in range(N_CORES)
    ]
    LAST_RESULT = run_bass_kernel_spmd(
        _NC_CACHE, in_maps, core_ids=list(range(N_CORES))
    )
    return np.concatenate([r["out"] for r in LAST_RESULT.results])


# revision 8
# speedup vs baseline: 1.0853x; 1.0464x over previous
"""Trainium2 Bass kernel for nn_Conv2DLayer_16011638080159.

Math: out = C * (x @ weight.sum(0))   with x [524288, 512], weight [9, 512].
Equivalent to a row-wise dot product of x with w_eff = C * weight.sum(0).

Strategy (pure data parallel, per sharding hint):
  - Shard x along the batch axis across 8 NeuronCores (65536 rows each).
  - Host-side prep: fold the K=9 weight sum and the C scale into a single
    [C] vector, replicated to a [128, C] SBUF constant (256 KiB, loaded once
    on the Activation ring so the SP ring starts streaming x immediately).
  - Per core: stream x in [128, 8*C] tiles (16 KiB descriptors - measured
    fastest: 32 KiB descriptors run 2x slower per DMA engine). Even tiles
    ride the SP HWDGE ring, odd tiles the Activation HWDGE ring. Measured
    pure-DMA floor for this config: ~403 us/core (334 GB/s, the per-core
    DMA fabric cap; queue count does not change it).
  - Compute: DVE only, one fused affine_mul_reduce per row slot
    (out=scratch, accum_out=o column): accum = sum((x*1+0)*w) does the
    multiply AND the row-sum in one 512-elem pass, measured 613 ns/slot
    pipelined (the 82 ns accumulator read overlaps the next op) ->
    4.9 us/tile, under the ~5.2-6.3 us/tile DMA cadence.
    (tensor_tensor_reduce and scalar_tensor_tensor+accum both fail this
    compiler/runtime; affine_mul_reduce is the custom-DVE path that works.)
    The Activation engine does NO compute (its ~1 us/slot accumulate chain
    was the baseline's critical path at 453 us busy).
  - Row mapping: shard row (p*512 + t*R + r) sits at partition p, tile t,
    slot r, so the per-core result tile [128, 512] is exactly the row-major
    view of the per-core output [65536]; one contiguous DMA writes it out.
"""

import numpy as np

import concourse.bacc as bacc
import concourse.bass as bass
import concourse.tile as tile
from concourse import mybir
from concourse.bass_utils import run_bass_kernel_spmd

B = 524288        # total rows
C = 512           # row length
N_CORES = 8
BS = B // N_CORES  # 65536 rows per core
P = 128            # SBUF partitions
RPP = BS // P      # 512 rows per partition
R = 8              # rows per partition per tile -> 16 KiB descriptors
F = R * C          # 4096 free elems per tile
NT = RPP // R      # 64 tiles per core
XS_BUFS = 8        # x tile pool depth (8 * 2 MiB = 16 MiB SBUF)

_NC_CACHE = None
LAST_RESULT = None  # BassKernelResults of the most recent run (for profiling)


def _build() -> bass.Bass:
    # Bacc (not raw Bass): its compile() pass splits multi-sem waits into
    # EventSemaphore instructions -- the TRN2 ISA allows only 1 wait/inst.
    nc = bacc.Bacc(None, target_bir_lowering=False, debug=False)
    x = nc.dram_tensor("x", [BS, C], mybir.dt.float32, kind="ExternalInput")
    w = nc.dram_tensor("w", [P, C], mybir.dt.float32, kind="ExternalInput")
    out = nc.dram_tensor("out", [BS], mybir.dt.float32, kind="ExternalOutput")

    # shard row (p*RPP + q) -> partition p, in-partition slot q; tiles are
    # [qs, qe) slot ranges. The last two tiles are half-size (R=4) so the
    # compute that trails the final DMA packet is halved.
    xp = x.rearrange("(p q) c -> p q c", p=P)
    ov = out.rearrange("(p f) -> p f", p=P)
    tiles = [(t * R, (t + 1) * R) for t in range(NT - 1)]
    tiles += [((NT - 1) * R, (NT - 1) * R + 4), ((NT - 1) * R + 4, NT * R)]

    with tile.TileContext(nc) as tc:
        with (
            tc.tile_pool(name="const", bufs=1) as cpool,
            tc.tile_pool(name="xs", bufs=XS_BUFS) as xs,
            tc.tile_pool(name="dmv", bufs=2) as dmv,
            tc.tile_pool(name="res", bufs=1) as res,
        ):
            w_t = cpool.tile([P, C], mybir.dt.float32)
            # weight rides the idle GpSimd SWDGE queue so BOTH HWDGE rings
            # start streaming x tiles immediately.
            nc.gpsimd.dma_start(out=w_t[:], in_=w[:, :])
            o_t = res.tile([P, RPP], mybir.dt.float32)
            # Output writeback is split: bulk chunks go out on the idle
            # GpSimd SWDGE as soon as their columns are done (Pool just
            # blocks on the sem - harmless), leaving only a tiny 16-column
            # chunk serialized after the last half-tile's compute.
            cut1 = (NT - 8) * R   # 448 cols, ready after tile 55
            cut2 = (NT - 1) * R   # 504 cols, ready after tile 62
            for t, (qs, qe) in enumerate(tiles):
                k = qe - qs
                x_t = xs.tile([P, k * C], mybir.dt.float32)
                if t % 2 == 0:
                    nc.sync.dma_start(out=x_t[:], in_=xp[:, qs:qe, :])
                else:
                    nc.scalar.dma_start(out=x_t[:], in_=xp[:, qs:qe, :])

                # DVE: fused (x*w) + row-sum, one slot per instruction
                for r in range(k):
                    col = qs + r
                    dm = dmv.tile([P, C], mybir.dt.float32, tag="dve_scr")
                    nc.vector.affine_mul_reduce(
                        out=dm[:],
                        accum_out=o_t[:, col: col + 1],
                        in0=x_t[:, r * C:(r + 1) * C],
                        in1=w_t[:],
                        scale=1.0,
                        bias=0.0,
                    )
                if qe == cut1:
                    nc.gpsimd.dma_start(out=ov[:, 0:cut1], in_=o_t[:, 0:cut1])
                if qe == cut2:
                    nc.gpsimd.dma_start(
                        out=ov[:, cut1:cut2], in_=o_t[:, cut1:cut2]
                    )
            nc.sync.dma_start(out=ov[:, cut2:RPP], in_=o_t[:, cut2:RPP])
    nc.finalize()
    return nc


def kernel(x: np.ndarray, weight: np.ndarray) -> np.ndarray:
    global _NC_CACHE, LAST_RESULT
    x = np.ascontiguousarray(np.asarray(x), dtype=np.float32)
    weight = np.asarray(weight, dtype=np.float32)

    w_eff = (C * weight.sum(axis=0)).astype(np.float32)   # [C]
    w_rep = np.ascontiguousarray(np.tile(w_eff, (P, 1)))  # [P, C]

    if _NC_CACHE is None:
        _NC_CACHE = _build()

    in_maps = [
        {"x": x[i * BS:(i + 1) * BS], "w": w_rep} for i in range(N_CORES)
    ]
    LAST_RESULT = run_bass_kernel_spmd(
        _NC_CACHE, in_maps, core_ids=list(range(N_CORES))
    )
    return np.concatenate([r["out"] for r in LAST_RESULT.results])


# revision 9
# speedup vs baseline: 1.1027x; 1.0160x over previous
"""Trainium2 Bass kernel for nn_Conv2DLayer_16011638080159.

Math: out = C * (x @ weight.sum(0))   with x [524288, 512], weight [9, 512].
Equivalent to a row-wise dot product of x with w_eff = C * weight.sum(0).

Strategy (pure data parallel, per sharding hint):
  - Shard x along the batch axis across 8 NeuronCores (65536 rows each).
  - Host-side prep: fold the K=9 weight sum and the C scale into a single
    [C] vector, replicated to a [128, C] SBUF constant (256 KiB, loaded once
    on the Activation ring so the SP ring starts streaming x immediately).
  - Per core: stream x in [128, 8*C] tiles (16 KiB descriptors - measured
    fastest: 32 KiB descriptors run 2x slower per DMA engine). Even tiles
    ride the SP HWDGE ring, odd tiles the Activation HWDGE ring. Measured
    pure-DMA floor for this config: ~403 us/core (334 GB/s, the per-core
    DMA fabric cap; queue count does not change it).
  - Compute: DVE only, one fused affine_mul_reduce per row slot
    (out=scratch, accum_out=o column): accum = sum((x*1+0)*w) does the
    multiply AND the row-sum in one 512-elem pass, measured 613 ns/slot
    pipelined (the 82 ns accumulator read overlaps the next op) ->
    4.9 us/tile, under the ~5.2-6.3 us/tile DMA cadence.
    (tensor_tensor_reduce and scalar_tensor_tensor+accum both fail this
    compiler/runtime; affine_mul_reduce is the custom-DVE path that works.)
    The Activation engine does NO compute (its ~1 us/slot accumulate chain
    was the baseline's critical path at 453 us busy).
  - Row mapping: shard row (p*512 + t*R + r) sits at partition p, tile t,
    slot r, so the per-core result tile [128, 512] is exactly the row-major
    view of the per-core output [65536]; one contiguous DMA writes it out.
"""

import numpy as np

import concourse.bacc as bacc
import concourse.bass as bass
import concourse.tile as tile
from concourse import mybir
from concourse.bass_utils import run_bass_kernel_spmd

B = 524288        # total rows
C = 512           # row length
N_CORES = 8
BS = B // N_CORES  # 65536 rows per core
P = 128            # SBUF partitions
RPP = BS // P      # 512 rows per partition
R = 8              # rows per partition per tile -> 16 KiB descriptors
F = R * C          # 4096 free elems per tile
NT = RPP // R      # 64 tiles per core
XS_BUFS = 8        # x tile pool depth (8 * 2 MiB = 16 MiB SBUF)

_NC_CACHE = None
LAST_RESULT = None  # BassKernelResults of the most recent run (for profiling)


def _build() -> bass.Bass:
    # Bacc (not raw Bass): its compile() pass splits multi-sem waits into
    # EventSemaphore instructions -- the TRN2 ISA allows only 1 wait/inst.
    nc = bacc.Bacc(None, target_bir_lowering=False, debug=False)
    x = nc.dram_tensor("x", [BS, C], mybir.dt.float32, kind="ExternalInput")
    w = nc.dram_tensor("w", [P, C], mybir.dt.float32, kind="ExternalInput")
    out = nc.dram_tensor("out", [BS], mybir.dt.float32, kind="ExternalOutput")

    # shard row (p*RPP + q) -> partition p, in-partition slot q; tiles are
    # [qs, qe) slot ranges. The last two tiles are half-size (R=4) so the
    # compute that trails the final DMA packet is halved.
    xp = x.rearrange("(p q) c -> p q c", p=P)
    ov = out.rearrange("(p f) -> p f", p=P)
    tiles = [(t * R, (t + 1) * R) for t in range(NT - 1)]
    tiles += [((NT - 1) * R, (NT - 1) * R + 4), ((NT - 1) * R + 4, NT * R)]

    with tile.TileContext(nc) as tc:
        with (
            tc.tile_pool(name="const", bufs=1) as cpool,
            tc.tile_pool(name="xs", bufs=XS_BUFS) as xs,
            tc.tile_pool(name="dmv", bufs=8) as dmv,
            tc.tile_pool(name="res", bufs=1) as res,
        ):
            w_t = cpool.tile([P, C], mybir.dt.float32)
            # weight rides the idle GpSimd SWDGE queue so BOTH HWDGE rings
            # start streaming x tiles immediately.
            nc.gpsimd.dma_start(out=w_t[:], in_=w[:, :])
            o_t = res.tile([P, RPP], mybir.dt.float32)
            # Output writeback is split: bulk chunks go out on the idle
            # GpSimd SWDGE as soon as their columns are done (Pool just
            # blocks on the sem - harmless), leaving only a tiny 16-column
            # chunk serialized after the last half-tile's compute.
            cut1 = (NT - 8) * R   # 448 cols, ready after tile 55
            cut2 = (NT - 1) * R   # 504 cols, ready after tile 62
            for t, (qs, qe) in enumerate(tiles):
                k = qe - qs
                x_t = xs.tile([P, k * C], mybir.dt.float32)
                if t % 2 == 0:
                    nc.sync.dma_start(out=x_t[:], in_=xp[:, qs:qe, :])
                else:
                    nc.scalar.dma_start(out=x_t[:], in_=xp[:, qs:qe, :])

                # DVE: fused (x*w) + row-sum, one slot per instruction
                for r in range(k):
                    col = qs + r
                    dm = dmv.tile([P, C], mybir.dt.float32, tag="dve_scr")
                    nc.vector.affine_mul_reduce(
                        out=dm[:],
                        accum_out=o_t[:, col: col + 1],
                        in0=x_t[:, r * C:(r + 1) * C],
                        in1=w_t[:],
                        scale=1.0,
                        bias=0.0,
                    )
                if qe == cut1:
                    nc.gpsimd.dma_start(out=ov[:, 0:cut1], in_=o_t[:, 0:cut1])
                if qe == cut2:
                    nc.gpsimd.dma_start(
                        out=ov[:, cut1:cut2], in_=o_t[:, cut1:cut2]
                    )
            nc.sync.dma_start(out=ov[:, cut2:RPP], in_=o_t[:, cut2:RPP])
    nc.finalize()
    return nc


def kernel(x: np.ndarray, weight: np.ndarray) -> np.ndarray:
    global _NC_CACHE, LAST_RESULT
    x = np.ascontiguousarray(np.asarray(x), dtype=np.float32)
    weight = np.asarray(weight, dtype=np.float32)

    w_eff = (C * weight.sum(axis=0)).astype(np.float32)   # [C]
    w_rep = np.ascontiguousarray(np.tile(w_eff, (P, 1)))  # [P, C]

    if _NC_CACHE is None:
        _NC_CACHE = _build()

    in_maps = [
        {"x": x[i * BS:(i + 1) * BS], "w": w_rep} for i in range(N_CORES)
    ]
    LAST_RESULT = run_bass_kernel_spmd(
        _NC_CACHE, in_maps, core_ids=list(range(N_CORES))
    )
    return np.concatenate([r["out"] for r in LAST_RESULT.results])
